# revision 8
# baseline (speedup 1.0000x reference)
"""2-layer GCN (PyG GCNConv semantics) on 8 Trainium2 NeuronCores.

Design (v2 — dma_gather based):
  - Nodes sharded contiguously across 8 cores (dst ownership), windows of
    128 dst nodes in ID ORDER (no degree sort, no indirect scatters).
  - Per layer, a table T (dis*(x@W1) resp. dis*h@W2, 64 f32 = 256B rows)
    is computed shard-locally and AllGather'd to t*f [8*ROWS, 64].
  - Per (window, quarter-of-table) the edges' source rows are fetched with
    dma_gather (int16 indices into a 25002-row quarter slice, <=1024 idx
    per call, rotating over 4 SWDGE queues) into a [128, NCH_w, 64] grid;
    padding slots point at the quarter's zero row.
  - The grid is summed in-place with a strided pairwise tree on the vector
    engine; epilogue applies dis[dst], bias, relu and stages results in
    persistent SBUF tiles; one strided DMA writes each layer's staging
    tile back to DRAM in node-id order.
  - The same int16 index stream drives both layers (same graph).

kernel(**inputs) takes FULL inputs, returns the FULL [N,40] output.
"""

import numpy as np
import ml_dtypes

import concourse.bass as bass
import concourse.bacc as bacc
import concourse.tile as tile
import concourse.mybir as mybir
from concourse.library_config import mlp as _mlp_lib

F32 = mybir.dt.float32
BF16 = mybir.dt.bfloat16
I16 = mybir.dt.int16

NCORES = 8
NQUART = 4
MAXCHUNK = 8          # chunks per dma_gather call (8*128 = 1024 idxs)
TW = 64               # table row width (f32) for both layers


def _round_up(x, m):
    return ((x + m - 1) // m) * m


def _prep(x, edge_index, W1, b1, W2, b2):
    """Host-side partitioning + metadata packing (numpy only)."""
    N, IN_DIM = x.shape
    HID = W1.shape[1]
    OUT = W2.shape[1]
    assert N % NCORES == 0
    SHARD = N // NCORES
    ROWS = SHARD + 1                   # + zero row per shard
    NT = _round_up(SHARD, 128) // 128  # id-ordered windows per core
    SHARD_PAD = NT * 128
    QROWS = 2 * ROWS                   # table rows per quarter
    assert QROWS - 1 <= 32767

    src = edge_index[0].astype(np.int64)
    dst = edge_index[1].astype(np.int64)
    loops = np.arange(N, dtype=np.int64)
    src = np.concatenate([src, loops])
    dst = np.concatenate([dst, loops])
    deg = np.bincount(dst, minlength=N).astype(np.float64)
    dis = (1.0 / np.sqrt(deg)).astype(np.float32)

    core_of = dst // SHARD
    dstloc = dst % SHARD
    srcloc = src % SHARD
    src_core = src // SHARD

    # Node->position permutation per core (rho-order): sorting nodes by their
    # worst-quarter in-degree flattens per-window-lane maxima, cutting the
    # gather grid padding ~40% vs id order. Table rows, windows, lanes, h~,
    # and the output all live in rho-order on device; the host unpermutes.
    grow0 = src_core * ROWS + srcloc
    q0 = grow0 // QROWS
    ranks = np.empty((NCORES, SHARD), np.int64)
    perms = np.empty((NCORES, SHARD), np.int64)
    for c in range(NCORES):
        m = core_of == c
        cq = np.zeros((SHARD, NQUART), np.int64)
        np.add.at(cq, (dstloc[m], q0[m]), 1)
        key = cq.max(1) * 100000 + deg[c * SHARD:(c + 1) * SHARD].astype(np.int64)
        o = np.argsort(-key, kind="stable")
        perms[c] = o
        ranks[c, o] = np.arange(SHARD)

    grow = src_core * ROWS + ranks[src_core, srcloc]  # rho-order table row
    q = grow // QROWS
    qloc = (grow % QROWS).astype(np.int64)
    pos = ranks[core_of, dstloc]
    w_of = pos // 128
    lane = pos % 128

    # per-(core,w,q,lane) counts -> K[w,q] = max over cores+lanes
    cnt = np.zeros((NCORES, NT, NQUART, 128), np.int64)
    np.add.at(cnt, (core_of, w_of, q, lane), 1)
    K = cnt.max(axis=(0, 3))           # [NT, NQUART]
    K = np.maximum(K, 1)
    NCHW = K.sum(axis=1)               # chunks per window
    coffq = np.zeros((NT, NQUART), np.int64)   # chunk col of quarter run in window
    coffq[:, 1:] = np.cumsum(K, axis=1)[:, :-1]
    coffw = np.concatenate([[0], np.cumsum(NCHW)]).astype(np.int64)  # window chunk base
    NCHT = int(coffw[-1])

    ZQ = SHARD                         # quarter-local zero row (shard 2q's)

    in_maps = []
    for c in range(NCORES):
        m = core_of == c
        o = np.argsort(w_of[m] * (NQUART * 128) + q[m] * 128 + lane[m], kind="stable")
        wm, qm, lm, vm = w_of[m][o], q[m][o], lane[m][o], qloc[m][o]
        key = wm * (NQUART * 128) + qm * 128 + lm
        first = np.searchsorted(key, key, side="left")
        ordinal = np.arange(len(key)) - first
        col = coffw[wm] + coffq[wm, qm] + ordinal    # global chunk column
        grid = np.full((128, NCHT), ZQ, dtype=np.int16)
        grid[lm, col] = vm.astype(np.int16)
        # chunk-major flatten -> [16, NCHT*8] (idx i at partition i%16, col i//16)
        arr = grid.T.ravel()
        idx16 = np.ascontiguousarray(arr.reshape(-1, 16).T)

        dis_pos = dis[c * SHARD + perms[c]]  # dis by device position
        disid = np.zeros((128, NT), np.float32)
        ids = np.arange(SHARD_PAD).reshape(NT, 128).T
        okm = ids < SHARD
        disid[okm] = dis_pos[ids[okm]]

        xT = np.zeros((IN_DIM, SHARD_PAD), dtype=ml_dtypes.bfloat16)
        xT[:, :SHARD] = x[c * SHARD + perms[c]].T.astype(ml_dtypes.bfloat16)

        W2p = np.zeros((128, TW), dtype=ml_dtypes.bfloat16)
        W2p[:HID, :OUT] = W2.astype(ml_dtypes.bfloat16)

        in_maps.append({
            "xT": xT,
            "idx16": idx16,
            "disid": disid,
            "W1": W1.astype(ml_dtypes.bfloat16),
            "W2p": W2p,
            "b1t": np.tile(np.asarray(b1, np.float32)[None, :], (128, 1)),
            "b2t": np.tile(np.asarray(b2, np.float32)[None, :], (128, 1)),
        })

    dims = dict(N=N, IN_DIM=IN_DIM, HID=HID, OUT=OUT, SHARD=SHARD, ROWS=ROWS,
                NT=NT, SHARD_PAD=SHARD_PAD, QROWS=QROWS, NCHT=NCHT, perms=perms)
    sched = dict(K=K.tolist(), NCHW=NCHW.tolist(),
                 coffq=coffq.tolist(), coffw=coffw.tolist())
    return in_maps, sched, dims


def _tree_sum_inplace(nc, g2d, n, F):
    """Sum n [128,F] chunks of g2d ([128, >=n*F]) into chunk 0, in place."""
    while n > 1:
        if n % 2 == 1:
            nc.vector.tensor_add(
                g2d[:, (n - 2) * F:(n - 1) * F],
                g2d[:, (n - 2) * F:(n - 1) * F],
                g2d[:, (n - 1) * F:n * F],
            )
            n -= 1
        h = n // 2
        a = g2d[:, :2 * h * F].rearrange("p (c f) -> p c f", f=2 * F)
        nc.vector.tensor_add(
            g2d[:, :h * F].rearrange("p (c f) -> p c f", f=F),
            a[:, :, :F],
            a[:, :, F:],
        )
        n = h


def _build(sched, d):
    HID, OUT, IN_DIM = d["HID"], d["OUT"], d["IN_DIM"]
    SHARD, ROWS, NT, SHARD_PAD = d["SHARD"], d["ROWS"], d["NT"], d["SHARD_PAD"]
    QROWS, NCHT = d["QROWS"], d["NCHT"]
    K, NCHW = sched["K"], sched["NCHW"]
    coffq, coffw = sched["coffq"], sched["coffw"]
    NCHMAX = max(NCHW)
    GMAX = max(coffw[min(w0 + 12, NT)] - coffw[w0] for w0 in range(0, NT, 12))
    HW2 = 128  # h~ row width (DMA transpose needs free %128)

    nc = bacc.Bacc("TRN2", target_bir_lowering=False, debug=False,
                   num_devices=NCORES, num_swdge_queues=4)
    xT_d = nc.dram_tensor("xT", [IN_DIM, SHARD_PAD], BF16, kind="ExternalInput")
    idx_d = nc.dram_tensor("idx16", [16, NCHT * 8], I16, kind="ExternalInput")
    disid_d = nc.dram_tensor("disid", [128, NT], F32, kind="ExternalInput")
    W1_d = nc.dram_tensor("W1", [IN_DIM, HID], BF16, kind="ExternalInput")
    W2p_d = nc.dram_tensor("W2p", [128, TW], BF16, kind="ExternalInput")
    b1t_d = nc.dram_tensor("b1t", [128, HID], F32, kind="ExternalInput")
    b2t_d = nc.dram_tensor("b2t", [128, OUT], F32, kind="ExternalInput")
    out_d = nc.dram_tensor("out", [SHARD, OUT], F32, kind="ExternalOutput")

    t1l = nc.dram_tensor("t1l", [ROWS, TW], F32, kind="Internal")
    t1f = nc.dram_tensor("t1f", [ROWS * NCORES, TW], F32, kind="Internal",
                         addr_space="Shared")
    hl = nc.dram_tensor("hl", [SHARD_PAD, HW2], BF16, kind="Internal")
    t2l = nc.dram_tensor("t2l", [ROWS, TW], F32, kind="Internal")
    t2f = nc.dram_tensor("t2f", [ROWS * NCORES, TW], F32, kind="Internal",
                         addr_space="Shared")
    rg = [list(range(NCORES))]

    qn = [0]  # rotating SWDGE queue

    GW = 12  # windows per idx-load group

    def gather_layer(tf, gat, idxp, meta, disid_sb, bias_sb, epilogue):
        """Per-window gathers + tree sum + epilogue(w, sum_ap)."""
        for w0 in range(0, NT, GW):
            w1 = min(w0 + GW, NT)
            gch = coffw[w1] - coffw[w0]
            idxg = idxp.tile([128, GMAX * 8], I16, tag="idxg")
            nc.sync.dma_start(
                out=idxg[:, :gch * 8],
                in_=idx_d[:, coffw[w0] * 8:(coffw[w0] + gch) * 8]
                .unsqueeze(0).broadcast_to([8, 16, gch * 8]),
            )
            for w in range(w0, w1):
                base = coffw[w] - coffw[w0]
                nch = NCHW[w]
                g = gat.tile([128, NCHMAX * TW], F32, tag="g")
                for q in range(NQUART):
                    c0 = coffq[w][q]
                    kq = K[w][q]
                    for s0 in range(0, kq, MAXCHUNK):
                        ncall = min(MAXCHUNK, kq - s0)
                        ni = ncall * 128
                        nc.gpsimd.dma_gather(
                            g[:, (c0 + s0) * TW:(c0 + s0 + ncall) * TW]
                            .rearrange("p (c f) -> p c f", f=TW),
                            tf[q * QROWS:(q + 1) * QROWS, :],
                            idxg[:, (base + c0 + s0) * 8:(base + c0 + s0 + ncall) * 8],
                            ni, ni, TW, queue_num=qn[0])
                        qn[0] = (qn[0] + 1) % 4
                _tree_sum_inplace(nc, g[:, :nch * TW], nch, TW)
                epilogue(w, g[:, :TW])

    with tile.TileContext(nc) as tc:
        with (
            tc.tile_pool(name="meta", bufs=1) as meta,
            tc.tile_pool(name="mm", bufs=3) as mm,
            tc.tile_pool(name="ps", bufs=4, space="PSUM") as ps,
            tc.tile_pool(name="gat", bufs=3) as gat,
            tc.tile_pool(name="idxp", bufs=2) as idxp,
            tc.tile_pool(name="epi", bufs=3) as epi,
        ):
            nc.gpsimd.load_library(_mlp_lib)

            xT_sb = meta.tile([IN_DIM, SHARD_PAD], BF16, tag="bigT")
            nc.sync.dma_start(out=xT_sb[:], in_=xT_d[:])
            disid_sb = meta.tile([128, NT], F32)
            nc.sync.dma_start(out=disid_sb[:], in_=disid_d[:])
            W1_sb = meta.tile([IN_DIM, HID], BF16)
            nc.sync.dma_start(out=W1_sb[:], in_=W1_d[:])
            W2p_sb = meta.tile([128, TW], BF16)
            nc.sync.dma_start(out=W2p_sb[:], in_=W2p_d[:])
            b1t_sb = meta.tile([128, HID], F32)
            nc.sync.dma_start(out=b1t_sb[:], in_=b1t_d[:])
            b2t_sb = meta.tile([128, OUT], F32)
            nc.sync.dma_start(out=b2t_sb[:], in_=b2t_d[:])
            zero_sb = meta.tile([128, TW], F32)
            nc.vector.memset(zero_sb[:], 0.0)

            # zero rows of the local tables
            nc.sync.dma_start(out=t1l[SHARD:SHARD + 1, :], in_=zero_sb[:1, :])
            nc.sync.dma_start(out=t2l[SHARD:SHARD + 1, :], in_=zero_sb[:1, :])

            hbig = meta.tile([128, NT * HW2], BF16, tag="hbig")
            nc.vector.memset(hbig[:], 0.0)
            obig = meta.tile([128, NT * OUT], F32, tag="obig")

            # ---- phase 1: T1 = dis * (x @ W1) ----
            for t in range(NT):
                p1 = ps.tile([128, HID], F32, tag="p1")
                nc.tensor.matmul(out=p1[:], lhsT=xT_sb[:, t * 128:(t + 1) * 128],
                                 rhs=W1_sb[:], start=True, stop=True)
                st = mm.tile([128, TW], F32, tag="st1")
                if TW > HID:
                    nc.vector.memset(st[:, HID:], 0.0)
                nc.vector.tensor_scalar(
                    out=st[:, :HID], in0=p1[:], scalar1=disid_sb[:, t:t + 1],
                    scalar2=None, op0=mybir.AluOpType.mult)
                hi = min((t + 1) * 128, SHARD) - t * 128
                nc.sync.dma_start(out=t1l[t * 128:t * 128 + hi, :], in_=st[:hi, :])

            nc.gpsimd.collective_compute(
                "AllGather", mybir.AluOpType.bypass, replica_groups=rg,
                ins=[t1l[:]], outs=[t1f[:]])

            # ---- layer 1 aggregation ----
            def epi1(w, s_ap):
                t_ = epi.tile([128, HID], F32, tag="t1e")
                nc.vector.tensor_scalar(
                    out=t_[:], in0=s_ap[:, :HID], scalar1=disid_sb[:, w:w + 1],
                    scalar2=None, op0=mybir.AluOpType.mult)
                nc.vector.tensor_add(t_[:], t_[:], b1t_sb[:])
                nc.vector.tensor_scalar(
                    out=hbig[:, w * HW2:w * HW2 + HID], in0=t_[:],
                    scalar1=0.0, scalar2=disid_sb[:, w:w + 1],
                    op0=mybir.AluOpType.max, op1=mybir.AluOpType.mult)

            gather_layer(t1f, gat, idxp, meta, disid_sb, b1t_sb, epi1)

            # h~ staging -> hl (rho order), then transpose for phase 3
            nc.sync.dma_start(
                out=hl[:, :].rearrange("(w b) f -> b w f", b=128),
                in_=hbig[:].rearrange("p (w f) -> p w f", f=HW2))
            hT_sb = meta.tile([HW2, SHARD_PAD], BF16, tag="bigT")
            nc.sync.dma_start(out=hT_sb[:], in_=hl[:], transpose=True)

            # ---- phase 3: T2 = h~ @ W2 ----
            for t in range(NT):
                p2 = ps.tile([128, TW], F32, tag="p2")
                nc.tensor.matmul(out=p2[:], lhsT=hT_sb[:, t * 128:(t + 1) * 128],
                                 rhs=W2p_sb[:], start=True, stop=True)
                st2 = mm.tile([128, TW], F32, tag="st2")
                nc.vector.tensor_copy(st2[:], p2[:])
                hi = min((t + 1) * 128, SHARD) - t * 128
                nc.sync.dma_start(out=t2l[t * 128:t * 128 + hi, :], in_=st2[:hi, :])

            nc.gpsimd.collective_compute(
                "AllGather", mybir.AluOpType.bypass, replica_groups=rg,
                ins=[t2l[:]], outs=[t2f[:]])

            # ---- layer 2 aggregation ----
            def epi2(w, s_ap):
                t_ = epi.tile([128, OUT], F32, tag="t2e")
                nc.vector.tensor_scalar(
                    out=t_[:], in0=s_ap[:, :OUT], scalar1=disid_sb[:, w:w + 1],
                    scalar2=None, op0=mybir.AluOpType.mult)
                nc.vector.tensor_add(
                    obig[:, w * OUT:(w + 1) * OUT], t_[:], b2t_sb[:])

            gather_layer(t2f, gat, idxp, meta, disid_sb, b2t_sb, epi2)

            # obig -> out (id order, tail-split)
            WFULL = SHARD // 128
            nc.sync.dma_start(
                out=out_d[:WFULL * 128, :].rearrange("(w b) f -> b w f", b=128),
                in_=obig[:, :WFULL * OUT].rearrange("p (w f) -> p w f", f=OUT))
            rem = SHARD - WFULL * 128
            if rem:
                nc.sync.dma_start(
                    out=out_d[WFULL * 128:, :],
                    in_=obig[:rem, WFULL * OUT:(WFULL + 1) * OUT])

    nc.compile()
    return nc


def kernel(x, edge_index, W1, b1, W2, b2):
    x = np.asarray(x)
    edge_index = np.asarray(edge_index)
    W1 = np.asarray(W1)
    b1 = np.asarray(b1)
    W2 = np.asarray(W2)
    b2 = np.asarray(b2)
    in_maps, sched, dims = _prep(x, edge_index, W1, b1, W2, b2)
    nc = _build(sched, dims)
    global LAST_EXEC_NS
    out, LAST_EXEC_NS = _run_device_resident(nc, in_maps)
    SHARD = dims["SHARD"]
    perms = dims["perms"]
    res = np.empty((dims["N"], dims["OUT"]), np.float32)
    for c in range(NCORES):
        res[c * SHARD + perms[c]] = out[c * SHARD:(c + 1) * SHARD]
    return res


LAST_EXEC_NS = -1


def _run_device_resident(nc, in_maps, timed_reps=12):
    """Run the NEFF once for outputs, then time repeat executions.

    One jit executable; inputs are device_put once (sharded across the 8
    cores), so timed calls measure NEFF execution + dispatch rather than
    per-call host<->device transfer.
    """
    import time as _time

    import jax
    import concourse.mybir as mb
    from concourse import bass2jax
    from jax.experimental.shard_map import shard_map
    from jax.sharding import Mesh, NamedSharding, PartitionSpec

    in_maps = [dict(m) for m in in_maps]
    if nc.partition_id_tensor is not None:
        pname = nc.partition_id_tensor.name
        for c, m in enumerate(in_maps):
            m.setdefault(pname, np.array([[c]], dtype=np.uint32))

    in_names, out_names, out_avals, zero_outs = [], [], [], []
    for alloc in nc.m.functions[0].allocations:
        if not isinstance(alloc, mb.MemoryLocationSet):
            continue
        name = alloc.memorylocations[0].name
        if alloc.kind == "ExternalInput":
            in_names.append(name)
        elif alloc.kind == "ExternalOutput":
            out_names.append(name)
            shape = tuple(alloc.tensor_shape)
            dtype = mb.dt.np(alloc.dtype)
            out_avals.append(jax.core.ShapedArray(shape, dtype))
            zero_outs.append(np.zeros(shape, dtype))
    n_params = len(in_names)
    all_names = in_names + out_names

    def _body(*args):
        return tuple(
            bass2jax._bass_exec_p.bind(
                *args,
                out_avals=tuple(out_avals),
                in_names=tuple(all_names),
                out_names=tuple(out_names),
                lowering_input_output_aliases=(),
                sim_require_finite=True,
                sim_require_nnan=True,
                nc=nc,
            )
        )

    devices = jax.devices()[:NCORES]
    mesh = Mesh(np.asarray(devices), ("core",))
    spec = PartitionSpec("core")
    f = jax.jit(
        shard_map(
            _body,
            mesh=mesh,
            in_specs=(spec,) * (n_params + len(out_names)),
            out_specs=(spec,) * len(out_names),
            check_rep=False,
        ),
        keep_unused=True,
    )
    sh = NamedSharding(mesh, spec)
    ops = [
        jax.device_put(
            np.concatenate([np.asarray(m[nm]) for m in in_maps], axis=0), sh
        )
        for nm in in_names
    ] + [
        jax.device_put(np.concatenate([z] * NCORES, axis=0), sh)
        for z in zero_outs
    ]
    outs = f(*ops)  # compile + correctness execution
    jax.block_until_ready(outs)
    result = np.asarray(outs[out_names.index("out")])
    best = None
    for _ in range(timed_reps):
        try:
            t0 = _time.perf_counter()
            outs = f(*ops)
            jax.block_until_ready(outs)
            dt = _time.perf_counter() - t0
        except Exception:
            break
        best = dt if best is None or dt < best else best
    if best is None:
        best = 1.0
    return result, int(best * 1e9)


# revision 9
# speedup vs baseline: 1.3197x; 1.3197x over previous
"""2-layer GCN (PyG GCNConv semantics) on 8 Trainium2 NeuronCores.

Design (v2 — dma_gather based):
  - Nodes sharded contiguously across 8 cores (dst ownership), windows of
    128 dst nodes in rho-order (per-core permutation sorting nodes by their
    worst-quarter in-degree — flattens per-lane chunk maxima, ~40% fewer
    gather slots than id order; the host unpermutes the output).
  - Per layer, a table T (dis*(x@W1) resp. dis*h@W2, 64 f32 = 256B rows)
    is computed shard-locally and AllGather'd to t*f [8*ROWS, 64].
  - Per (window, quarter-of-table) the edges' source rows are fetched with
    dma_gather (int16 indices into a 25002-row quarter slice, <=1024 idx
    per call, rotating over 4 SWDGE queues) into a [128, NCH_w, 64] grid;
    padding slots point at the quarter's zero row.
  - The grid is summed in-place with a strided pairwise tree on the vector
    engine; epilogue applies dis[dst], bias, relu and stages results in
    persistent SBUF tiles; one strided DMA writes each layer's staging
    tile back to DRAM in rho order; no indirect scatters anywhere.
  - The same int16 index stream drives both layers (same graph).

kernel(**inputs) takes FULL inputs, returns the FULL [N,40] output.
"""

import numpy as np
import ml_dtypes

import concourse.bass as bass
import concourse.bacc as bacc
import concourse.tile as tile
import concourse.mybir as mybir
from concourse.library_config import mlp as _mlp_lib

F32 = mybir.dt.float32
BF16 = mybir.dt.bfloat16
I16 = mybir.dt.int16

NCORES = 8
NQUART = 4
MAXCHUNK = 8          # chunks per dma_gather call (8*128 = 1024 idxs)
TW = 64               # table row width (f32) for both layers


def _round_up(x, m):
    return ((x + m - 1) // m) * m


def _prep(x, edge_index, W1, b1, W2, b2):
    """Host-side partitioning + metadata packing (numpy only)."""
    N, IN_DIM = x.shape
    HID = W1.shape[1]
    OUT = W2.shape[1]
    assert N % NCORES == 0
    SHARD = N // NCORES
    ROWS = SHARD + 1                   # + zero row per shard
    NT = _round_up(SHARD, 128) // 128  # id-ordered windows per core
    SHARD_PAD = NT * 128
    QROWS = 2 * ROWS                   # table rows per quarter
    assert QROWS - 1 <= 32767

    src = edge_index[0].astype(np.int64)
    dst = edge_index[1].astype(np.int64)
    loops = np.arange(N, dtype=np.int64)
    src = np.concatenate([src, loops])
    dst = np.concatenate([dst, loops])
    deg = np.bincount(dst, minlength=N).astype(np.float64)
    dis = (1.0 / np.sqrt(deg)).astype(np.float32)

    core_of = dst // SHARD
    dstloc = dst % SHARD
    srcloc = src % SHARD
    src_core = src // SHARD

    # Node->position permutation per core (rho-order): sorting nodes by their
    # worst-quarter in-degree flattens per-window-lane maxima, cutting the
    # gather grid padding ~40% vs id order. Table rows, windows, lanes, h~,
    # and the output all live in rho-order on device; the host unpermutes.
    grow0 = src_core * ROWS + srcloc
    q0 = grow0 // QROWS
    ranks = np.empty((NCORES, SHARD), np.int64)
    perms = np.empty((NCORES, SHARD), np.int64)
    for c in range(NCORES):
        m = core_of == c
        cq = np.zeros((SHARD, NQUART), np.int64)
        np.add.at(cq, (dstloc[m], q0[m]), 1)
        key = cq.max(1) * 100000 + deg[c * SHARD:(c + 1) * SHARD].astype(np.int64)
        o = np.argsort(-key, kind="stable")
        perms[c] = o
        ranks[c, o] = np.arange(SHARD)

    grow = src_core * ROWS + ranks[src_core, srcloc]  # rho-order table row
    q = grow // QROWS
    qloc = (grow % QROWS).astype(np.int64)
    pos = ranks[core_of, dstloc]
    w_of = pos // 128
    lane = pos % 128

    # per-(core,w,q,lane) counts -> K[w,q] = max over cores+lanes
    cnt = np.zeros((NCORES, NT, NQUART, 128), np.int64)
    np.add.at(cnt, (core_of, w_of, q, lane), 1)
    K = cnt.max(axis=(0, 3))           # [NT, NQUART]
    K = np.maximum(K, 1)
    NCHW = K.sum(axis=1)               # chunks per window
    coffq = np.zeros((NT, NQUART), np.int64)   # chunk col of quarter run in window
    coffq[:, 1:] = np.cumsum(K, axis=1)[:, :-1]
    coffw = np.concatenate([[0], np.cumsum(NCHW)]).astype(np.int64)  # window chunk base
    NCHT = int(coffw[-1])

    ZQ = SHARD                         # quarter-local zero row (shard 2q's)

    in_maps = []
    for c in range(NCORES):
        m = core_of == c
        o = np.argsort(w_of[m] * (NQUART * 128) + q[m] * 128 + lane[m], kind="stable")
        wm, qm, lm, vm = w_of[m][o], q[m][o], lane[m][o], qloc[m][o]
        key = wm * (NQUART * 128) + qm * 128 + lm
        first = np.searchsorted(key, key, side="left")
        ordinal = np.arange(len(key)) - first
        col = coffw[wm] + coffq[wm, qm] + ordinal    # global chunk column
        grid = np.full((128, NCHT), ZQ, dtype=np.int16)
        grid[lm, col] = vm.astype(np.int16)
        # chunk-major flatten -> [16, NCHT*8] (idx i at partition i%16, col i//16)
        arr = grid.T.ravel()
        idx16 = np.ascontiguousarray(arr.reshape(-1, 16).T)

        dis_pos = dis[c * SHARD + perms[c]]  # dis by device position
        disid = np.zeros((128, NT), np.float32)
        ids = np.arange(SHARD_PAD).reshape(NT, 128).T
        okm = ids < SHARD
        disid[okm] = dis_pos[ids[okm]]

        xT = np.zeros((IN_DIM, SHARD_PAD), dtype=ml_dtypes.bfloat16)
        xT[:, :SHARD] = x[c * SHARD + perms[c]].T.astype(ml_dtypes.bfloat16)

        W2p = np.zeros((128, TW), dtype=ml_dtypes.bfloat16)
        W2p[:HID, :OUT] = W2.astype(ml_dtypes.bfloat16)

        in_maps.append({
            "xT": xT,
            "idx16": idx16,
            "disid": disid,
            "W1": W1.astype(ml_dtypes.bfloat16),
            "W2p": W2p,
            "b1t": np.tile(np.asarray(b1, np.float32)[None, :], (128, 1)),
            "b2t": np.tile(np.asarray(b2, np.float32)[None, :], (128, 1)),
        })

    dims = dict(N=N, IN_DIM=IN_DIM, HID=HID, OUT=OUT, SHARD=SHARD, ROWS=ROWS,
                NT=NT, SHARD_PAD=SHARD_PAD, QROWS=QROWS, NCHT=NCHT, perms=perms)
    sched = dict(K=K.tolist(), NCHW=NCHW.tolist(),
                 coffq=coffq.tolist(), coffw=coffw.tolist())
    return in_maps, sched, dims


def _tree_sum_inplace(nc, g2d, n, F):
    """Sum n [128,F] chunks of g2d ([128, >=n*F]) into chunk 0, in place."""
    while n > 1:
        if n % 2 == 1:
            nc.vector.tensor_add(
                g2d[:, (n - 2) * F:(n - 1) * F],
                g2d[:, (n - 2) * F:(n - 1) * F],
                g2d[:, (n - 1) * F:n * F],
            )
            n -= 1
        h = n // 2
        a = g2d[:, :2 * h * F].rearrange("p (c f) -> p c f", f=2 * F)
        nc.vector.tensor_add(
            g2d[:, :h * F].rearrange("p (c f) -> p c f", f=F),
            a[:, :, :F],
            a[:, :, F:],
        )
        n = h


def _build(sched, d):
    HID, OUT, IN_DIM = d["HID"], d["OUT"], d["IN_DIM"]
    SHARD, ROWS, NT, SHARD_PAD = d["SHARD"], d["ROWS"], d["NT"], d["SHARD_PAD"]
    QROWS, NCHT = d["QROWS"], d["NCHT"]
    K, NCHW = sched["K"], sched["NCHW"]
    coffq, coffw = sched["coffq"], sched["coffw"]
    NCHMAX = max(NCHW)
    GMAX = max(coffw[min(w0 + 12, NT)] - coffw[w0] for w0 in range(0, NT, 12))
    HW2 = 128  # h~ row width (DMA transpose needs free %128)

    nc = bacc.Bacc("TRN2", target_bir_lowering=False, debug=False,
                   num_devices=NCORES, num_swdge_queues=4)
    xT_d = nc.dram_tensor("xT", [IN_DIM, SHARD_PAD], BF16, kind="ExternalInput")
    idx_d = nc.dram_tensor("idx16", [16, NCHT * 8], I16, kind="ExternalInput")
    disid_d = nc.dram_tensor("disid", [128, NT], F32, kind="ExternalInput")
    W1_d = nc.dram_tensor("W1", [IN_DIM, HID], BF16, kind="ExternalInput")
    W2p_d = nc.dram_tensor("W2p", [128, TW], BF16, kind="ExternalInput")
    b1t_d = nc.dram_tensor("b1t", [128, HID], F32, kind="ExternalInput")
    b2t_d = nc.dram_tensor("b2t", [128, OUT], F32, kind="ExternalInput")
    out_d = nc.dram_tensor("out", [SHARD, OUT], F32, kind="ExternalOutput")

    t1l = nc.dram_tensor("t1l", [ROWS, TW], F32, kind="Internal")
    t1f = nc.dram_tensor("t1f", [ROWS * NCORES, TW], F32, kind="Internal",
                         addr_space="Shared")
    hl = nc.dram_tensor("hl", [SHARD_PAD, HW2], BF16, kind="Internal")
    t2l = nc.dram_tensor("t2l", [ROWS, TW], F32, kind="Internal")
    t2f = nc.dram_tensor("t2f", [ROWS * NCORES, TW], F32, kind="Internal",
                         addr_space="Shared")
    rg = [list(range(NCORES))]

    qn = [0]  # rotating SWDGE queue

    GW = 12  # windows per idx-load group

    def gather_layer(tf, gat, idxp, meta, disid_sb, bias_sb, epilogue):
        """Per-window gathers + tree sum + epilogue(w, sum_ap)."""
        for w0 in range(0, NT, GW):
            w1 = min(w0 + GW, NT)
            gch = coffw[w1] - coffw[w0]
            idxg = idxp.tile([128, GMAX * 8], I16, tag="idxg")
            nc.sync.dma_start(
                out=idxg[:, :gch * 8],
                in_=idx_d[:, coffw[w0] * 8:(coffw[w0] + gch) * 8]
                .unsqueeze(0).broadcast_to([8, 16, gch * 8]),
            )
            for w in range(w0, w1):
                base = coffw[w] - coffw[w0]
                nch = NCHW[w]
                g = gat.tile([128, NCHMAX * TW], F32, tag="g")
                for q in range(NQUART):
                    c0 = coffq[w][q]
                    kq = K[w][q]
                    for s0 in range(0, kq, MAXCHUNK):
                        ncall = min(MAXCHUNK, kq - s0)
                        ni = ncall * 128
                        nc.gpsimd.dma_gather(
                            g[:, (c0 + s0) * TW:(c0 + s0 + ncall) * TW]
                            .rearrange("p (c f) -> p c f", f=TW),
                            tf[q * QROWS:(q + 1) * QROWS, :],
                            idxg[:, (base + c0 + s0) * 8:(base + c0 + s0 + ncall) * 8],
                            ni, ni, TW, queue_num=qn[0])
                        qn[0] = (qn[0] + 1) % 4
                _tree_sum_inplace(nc, g[:, :nch * TW], nch, TW)
                epilogue(w, g[:, :TW])

    with tile.TileContext(nc) as tc:
        with (
            tc.tile_pool(name="meta", bufs=1) as meta,
            tc.tile_pool(name="mm", bufs=3) as mm,
            tc.tile_pool(name="ps", bufs=4, space="PSUM") as ps,
            tc.tile_pool(name="gat", bufs=3) as gat,
            tc.tile_pool(name="idxp", bufs=2) as idxp,
            tc.tile_pool(name="epi", bufs=3) as epi,
        ):
            nc.gpsimd.load_library(_mlp_lib)

            xT_sb = meta.tile([IN_DIM, SHARD_PAD], BF16, tag="bigT")
            nc.sync.dma_start(out=xT_sb[:], in_=xT_d[:])
            disid_sb = meta.tile([128, NT], F32)
            nc.sync.dma_start(out=disid_sb[:], in_=disid_d[:])
            W1_sb = meta.tile([IN_DIM, HID], BF16)
            nc.sync.dma_start(out=W1_sb[:], in_=W1_d[:])
            W2p_sb = meta.tile([128, TW], BF16)
            nc.sync.dma_start(out=W2p_sb[:], in_=W2p_d[:])
            b1t_sb = meta.tile([128, HID], F32)
            nc.sync.dma_start(out=b1t_sb[:], in_=b1t_d[:])
            b2t_sb = meta.tile([128, OUT], F32)
            nc.sync.dma_start(out=b2t_sb[:], in_=b2t_d[:])
            zero_sb = meta.tile([128, TW], F32)
            nc.vector.memset(zero_sb[:], 0.0)

            # zero rows of the local tables
            nc.sync.dma_start(out=t1l[SHARD:SHARD + 1, :], in_=zero_sb[:1, :])
            nc.sync.dma_start(out=t2l[SHARD:SHARD + 1, :], in_=zero_sb[:1, :])

            hbig = meta.tile([128, NT * HW2], BF16, tag="hbig")
            nc.vector.memset(hbig[:], 0.0)
            obig = meta.tile([128, NT * OUT], F32, tag="obig")

            # ---- phase 1: T1 = dis * (x @ W1) ----
            for t in range(NT):
                p1 = ps.tile([128, HID], F32, tag="p1")
                nc.tensor.matmul(out=p1[:], lhsT=xT_sb[:, t * 128:(t + 1) * 128],
                                 rhs=W1_sb[:], start=True, stop=True)
                st = mm.tile([128, TW], F32, tag="st1")
                if TW > HID:
                    nc.vector.memset(st[:, HID:], 0.0)
                nc.vector.tensor_scalar(
                    out=st[:, :HID], in0=p1[:], scalar1=disid_sb[:, t:t + 1],
                    scalar2=None, op0=mybir.AluOpType.mult)
                hi = min((t + 1) * 128, SHARD) - t * 128
                nc.sync.dma_start(out=t1l[t * 128:t * 128 + hi, :], in_=st[:hi, :])

            nc.gpsimd.collective_compute(
                "AllGather", mybir.AluOpType.bypass, replica_groups=rg,
                ins=[t1l[:]], outs=[t1f[:]])

            # ---- layer 1 aggregation ----
            def epi1(w, s_ap):
                t_ = epi.tile([128, HID], F32, tag="t1e")
                nc.vector.tensor_scalar(
                    out=t_[:], in0=s_ap[:, :HID], scalar1=disid_sb[:, w:w + 1],
                    scalar2=None, op0=mybir.AluOpType.mult)
                nc.vector.tensor_add(t_[:], t_[:], b1t_sb[:])
                nc.vector.tensor_scalar(
                    out=hbig[:, w * HW2:w * HW2 + HID], in0=t_[:],
                    scalar1=0.0, scalar2=disid_sb[:, w:w + 1],
                    op0=mybir.AluOpType.max, op1=mybir.AluOpType.mult)

            gather_layer(t1f, gat, idxp, meta, disid_sb, b1t_sb, epi1)

            # h~ staging -> hl (rho order), then transpose for phase 3
            nc.sync.dma_start(
                out=hl[:, :].rearrange("(w b) f -> b w f", b=128),
                in_=hbig[:].rearrange("p (w f) -> p w f", f=HW2))
            hT_sb = meta.tile([HW2, SHARD_PAD], BF16, tag="bigT")
            nc.sync.dma_start(out=hT_sb[:], in_=hl[:], transpose=True)

            # ---- phase 3: T2 = h~ @ W2 ----
            for t in range(NT):
                p2 = ps.tile([128, TW], F32, tag="p2")
                nc.tensor.matmul(out=p2[:], lhsT=hT_sb[:, t * 128:(t + 1) * 128],
                                 rhs=W2p_sb[:], start=True, stop=True)
                st2 = mm.tile([128, TW], F32, tag="st2")
                nc.vector.tensor_copy(st2[:], p2[:])
                hi = min((t + 1) * 128, SHARD) - t * 128
                nc.sync.dma_start(out=t2l[t * 128:t * 128 + hi, :], in_=st2[:hi, :])

            nc.gpsimd.collective_compute(
                "AllGather", mybir.AluOpType.bypass, replica_groups=rg,
                ins=[t2l[:]], outs=[t2f[:]])

            # ---- layer 2 aggregation ----
            def epi2(w, s_ap):
                t_ = epi.tile([128, OUT], F32, tag="t2e")
                nc.vector.tensor_scalar(
                    out=t_[:], in0=s_ap[:, :OUT], scalar1=disid_sb[:, w:w + 1],
                    scalar2=None, op0=mybir.AluOpType.mult)
                nc.vector.tensor_add(
                    obig[:, w * OUT:(w + 1) * OUT], t_[:], b2t_sb[:])

            gather_layer(t2f, gat, idxp, meta, disid_sb, b2t_sb, epi2)

            # obig -> out (id order, tail-split)
            WFULL = SHARD // 128
            nc.sync.dma_start(
                out=out_d[:WFULL * 128, :].rearrange("(w b) f -> b w f", b=128),
                in_=obig[:, :WFULL * OUT].rearrange("p (w f) -> p w f", f=OUT))
            rem = SHARD - WFULL * 128
            if rem:
                nc.sync.dma_start(
                    out=out_d[WFULL * 128:, :],
                    in_=obig[:rem, WFULL * OUT:(WFULL + 1) * OUT])

    nc.compile()
    return nc


def kernel(x, edge_index, W1, b1, W2, b2):
    x = np.asarray(x)
    edge_index = np.asarray(edge_index)
    W1 = np.asarray(W1)
    b1 = np.asarray(b1)
    W2 = np.asarray(W2)
    b2 = np.asarray(b2)
    in_maps, sched, dims = _prep(x, edge_index, W1, b1, W2, b2)
    nc = _build(sched, dims)
    global LAST_EXEC_NS
    out, LAST_EXEC_NS = _run_device_resident(nc, in_maps)
    SHARD = dims["SHARD"]
    perms = dims["perms"]
    res = np.empty((dims["N"], dims["OUT"]), np.float32)
    for c in range(NCORES):
        res[c * SHARD + perms[c]] = out[c * SHARD:(c + 1) * SHARD]
    return res


LAST_EXEC_NS = -1


def _run_device_resident(nc, in_maps, timed_reps=12):
    """Run the NEFF once for outputs, then time repeat executions.

    One jit executable; inputs are device_put once (sharded across the 8
    cores), so timed calls measure NEFF execution + dispatch rather than
    per-call host<->device transfer.
    """
    import time as _time

    import jax
    import concourse.mybir as mb
    from concourse import bass2jax
    from jax.experimental.shard_map import shard_map
    from jax.sharding import Mesh, NamedSharding, PartitionSpec

    in_maps = [dict(m) for m in in_maps]
    if nc.partition_id_tensor is not None:
        pname = nc.partition_id_tensor.name
        for c, m in enumerate(in_maps):
            m.setdefault(pname, np.array([[c]], dtype=np.uint32))

    in_names, out_names, out_avals, zero_outs = [], [], [], []
    for alloc in nc.m.functions[0].allocations:
        if not isinstance(alloc, mb.MemoryLocationSet):
            continue
        name = alloc.memorylocations[0].name
        if alloc.kind == "ExternalInput":
            in_names.append(name)
        elif alloc.kind == "ExternalOutput":
            out_names.append(name)
            shape = tuple(alloc.tensor_shape)
            dtype = mb.dt.np(alloc.dtype)
            out_avals.append(jax.core.ShapedArray(shape, dtype))
            zero_outs.append(np.zeros(shape, dtype))
    n_params = len(in_names)
    all_names = in_names + out_names

    def _body(*args):
        return tuple(
            bass2jax._bass_exec_p.bind(
                *args,
                out_avals=tuple(out_avals),
                in_names=tuple(all_names),
                out_names=tuple(out_names),
                lowering_input_output_aliases=(),
                sim_require_finite=True,
                sim_require_nnan=True,
                nc=nc,
            )
        )

    devices = jax.devices()[:NCORES]
    mesh = Mesh(np.asarray(devices), ("core",))
    spec = PartitionSpec("core")
    f = jax.jit(
        shard_map(
            _body,
            mesh=mesh,
            in_specs=(spec,) * (n_params + len(out_names)),
            out_specs=(spec,) * len(out_names),
            check_rep=False,
        ),
        keep_unused=True,
    )
    sh = NamedSharding(mesh, spec)
    ops = [
        jax.device_put(
            np.concatenate([np.asarray(m[nm]) for m in in_maps], axis=0), sh
        )
        for nm in in_names
    ] + [
        jax.device_put(np.concatenate([z] * NCORES, axis=0), sh)
        for z in zero_outs
    ]
    outs = f(*ops)  # compile + correctness execution
    jax.block_until_ready(outs)
    result = np.asarray(outs[out_names.index("out")])
    best = None
    for _ in range(timed_reps):
        try:
            t0 = _time.perf_counter()
            outs = f(*ops)
            jax.block_until_ready(outs)
            dt = _time.perf_counter() - t0
        except Exception:
            break
        best = dt if best is None or dt < best else best
    if best is None:
        best = 1.0
    return result, int(best * 1e9)
